# revision 45
# baseline (speedup 1.0000x reference)
"""BitLinear forward on 8 Trainium2 NeuronCores.

Reference computation (see harness reference.py):
    xn      = rmsnorm(x) * norm_weight                     # per token over D
    w_scale = 1 / max(mean(|W|), 1e-5)                     # global scalar
    w_q     = clip(round(W * w_scale), -1, 1)              # ternary
    x_scale = 127 / max(max|xn| per token, 1e-5)
    x_q     = clip(round(xn * x_scale), -128, 127)
    y       = (x_q @ w_q.T) / (w_scale * x_scale)

Distribution: data-parallel over tokens (1024/core), weight REPLICATED.
The host passes W pre-transposed (wt = W.T, [D, N] row-major) to every
core, so each core:
  - computes the global |W| abs-sum from its own disjoint 512-row slice
    of wt, AllReduces the scalar (the only collective on the critical
    path; a tiny warm-up AllReduce is issued at kernel start so the
    real one doesn't pay collective cold-start / launch skew),
  - rmsnorms + int8-quantizes + PE-transposes its 1024 tokens into a
    resident xqT [128, 32dc, 1024tok] bf16 SBUF tile, all before the
    matmul phase starts - no mid-matmul x stalls,
  - streams wt from its own DRAM in [128, 4dc, 512out] fp32 stages,
    quantizing to ternary bf16 on the fly (DVE round via the fp32
    +1.5*2^23 trick, then ACT Sign: clip(round(v),-1,1) == sign of the
    rounded integer; all ACT funcs used live in one table set),
  - runs 8 out-chunks x 8 token-tiles x 32 accumulating bf16 matmuls
    (exact integer arithmetic: x_q in [-127,127], w_q in {-1,0,1},
    fp32 PSUM partial sums < 2^24).
No AllGather, no quantized-W DRAM round-trip.
"""

import numpy as np

# ---------------------------------------------------------------- constants
R = 8  # cores
B, S, D = 4, 2048, 4096
N = 4096  # out features
TOK = (B * S) // R  # tokens per core (1024)
NS = N // R  # wslice rows per core (512)
HALF = D // 2  # x free-dim half tile (2048)
NTT = TOK // 128  # token tiles per core (8)
NDC = D // 128  # contraction chunks (32)
NCH = N // 512  # output chunks (8)
NST = 8  # W stages per chunk (4 dc each)
SDC = NDC // NST  # dc per stage (4)
C_ROUND = 12582912.0  # 1.5 * 2^23: fp32 add rounds to int, ties-to-even
EPS_NORM = 1e-5
Q_EPS = 1e-5

_CACHED = {}


def _legalize_waits(bir_bytes):
    """Split multi-wait BIR instructions into single-wait EventSemaphore
    chains: the walrus build here accepts at most one sync-wait command per
    instruction, while Tile's sem-assignment emits multi-wait joins."""
    import json

    bir = json.loads(bir_bytes)
    for fn in bir.get("functions", []):
        for bb in fn.get("blocks", []):
            new_insts = []
            for inst in bb.get("instructions", []):
                si = inst.get("sync_info")
                waits = (si or {}).get("on_wait") or []
                if len(waits) > 1:
                    movable = [w for w in waits if w.get("sync_type") == "semaphore"]
                    fixed = [w for w in waits if w.get("sync_type") != "semaphore"]
                    keep, hoist = (
                        (fixed, movable) if fixed else ([movable[-1]], movable[:-1])
                    )
                    if len(keep) > 1:
                        raise RuntimeError(
                            f"{inst.get('name')}: {len(keep)} non-hoistable waits"
                        )
                    for k, w in enumerate(hoist):
                        new_insts.append(
                            {
                                "debug": inst.get("debug", 0),
                                "engine": inst["engine"],
                                "ins": [],
                                "name": f"{inst['name']}_hw{k}",
                                "opcode": "EventSemaphore",
                                "outs": [],
                                "sync_info": {"on_update": [], "on_wait": [w]},
                            }
                        )
                    si["on_wait"] = keep
                new_insts.append(inst)
            bb["instructions"] = new_insts
    return json.dumps(bir).encode()


def _build(with_g):
    import concourse.bass as bass
    import concourse.mybir as mybir
    import concourse.tile as tile
    from concourse.bass import ts
    from concourse.masks import make_identity

    f32 = mybir.dt.float32
    bf16 = mybir.dt.bfloat16
    MULT = mybir.AluOpType.mult
    ADD = mybir.AluOpType.add
    MAX = mybir.AluOpType.max
    X_AX = mybir.AxisListType.X
    AF = mybir.ActivationFunctionType
    GROUP = [list(range(R))]

    nc = bass.Bass()
    xp = nc.declare_dram_parameter("x", [TOK, D], f32, isOutput=False)
    wtp = nc.declare_dram_parameter("wt", [D, N], f32, isOutput=False)
    wsl = nc.declare_dram_parameter("wsl", [NS, D], f32, isOutput=False)
    gp = nc.declare_dram_parameter("g", [1, D], f32, isOutput=False)
    yp = nc.declare_dram_parameter("y", [TOK, N], f32, isOutput=True)

    wcb_bufs = 12 if with_g else 16
    iox_bufs = 2 if with_g else 4
    scr_bufs = 4 if with_g else 2
    # software-pipeline depth of the x-prep: how many stats blocks are
    # emitted ahead of the trailing round/transpose blocks
    xpre = 1 if with_g else 2

    with tile.TileContext(nc) as tc:
        with (
            tc.tile_pool(name="persist", bufs=1) as pp,
            tc.tile_pool(name="io_x", bufs=iox_bufs) as io_x,
            tc.tile_pool(name="scr_x", bufs=scr_bufs) as scr_x,
            tc.tile_pool(name="wst", bufs=2) as wst,
            tc.tile_pool(name="wcb", bufs=wcb_bufs) as wcb_pool,
            tc.tile_pool(name="small", bufs=24) as sp,
            tc.tile_pool(name="yout", bufs=2) as ypool,
            tc.tile_pool(name="ps_t", bufs=2, space="PSUM") as ps_t,
            tc.tile_pool(name="ps_mm", bufs=6, space="PSUM") as ps_mm,
            tc.tile_pool(name="dram", bufs=1, space="DRAM") as dram,
        ):
            # ---- persistent tiles
            xqT = pp.tile([128, NDC, TOK], bf16, name="xqT")
            ident = pp.tile([128, 128], f32, name="ident")
            make_identity(nc, ident[:])
            ones_col = pp.tile([128, 1], f32, name="ones_col")
            nc.vector.memset(ones_col[:], 1.0)
            amc_all = pp.tile([128, NTT], f32, name="amc_all")
            c2_all = pp.tile([128, NTT], f32, name="c2_all")
            c_all = pp.tile([128, NTT], f32, name="c_all")
            s_rep = pp.tile([128, 1], f32, name="s_rep")
            m_rep = pp.tile([128, 1], f32, name="m_rep")
            wsc_rep = pp.tile([128, 1], f32, name="wsc_rep")
            dq_rep = pp.tile([128, 1], f32, name="dq_rep")
            eps_rep = pp.tile([128, 1], f32, name="eps_rep")
            nc.vector.memset(eps_rep[:], EPS_NORM)
            nround_rep = pp.tile([128, 1], f32, name="nround_rep")
            nc.vector.memset(nround_rep[:], -C_ROUND)
            if with_g:
                g_rep = pp.tile([128, D], f32, name="g_rep")
                nc.sync.dma_start(g_rep[:], gp[:].to_broadcast([128, D]))

            # ---- DRAM scratch
            ws_in = dram.tile([1, 1], f32, name="ws_in")
            ws_out = dram.tile([1, 1], f32, addr_space="Shared", name="ws_out")

            # ---- W1: partial |W| abs-sum over this core's disjoint slice
            parts = []

            def emit_w1(i, h):
                w_t = wst.tile([128, SDC, 512], f32, tag="wst", name="ws_t")
                fl = w_t[:].rearrange("p j q -> p (j q)")
                nc.sync.dma_start(fl, wsl[ts(i, 128), ts(h, HALF)])
                part = sp.tile([128, 1], f32, tag="sm", name="part")
                nc.scalar.activation(fl, fl, AF.Abs, accum_out=part[:])
                parts.append(part)

            # ---- X(t): rmsnorm + int8 quantize + transpose into xqT.
            # Split into a stats part and a round/transpose part so the
            # emission loop can software-pipeline them (stats of tile t+1
            # ahead of round of tile t in every engine FIFO).
            x_state = {}

            def emit_x_stats(t):
                srcs = []
                mss, amaxs = [], []
                for h in range(2):
                    x_t = io_x.tile([128, HALF], f32, tag="iox", name="x_t")
                    nc.sync.dma_start(x_t[:], xp[ts(t, 128), ts(h, HALF)])
                    ms_h = sp.tile([128, 1], f32, tag="sm", name="ms_h")
                    sq = scr_x.tile([128, HALF], f32, tag="scx", name="sq")
                    # sq <- x*x (junk scratch), ms_h <- sum(x^2)
                    nc.scalar.activation(sq[:], x_t[:], AF.Square, accum_out=ms_h[:])
                    if with_g:
                        nc.vector.tensor_mul(sq[:], x_t[:], g_rep[:, ts(h, HALF)])
                        src = sq
                    else:
                        src = x_t
                    srcs.append(src)
                    am_h = sp.tile([128, 1], f32, tag="sm", name="am_h")
                    nc.vector.tensor_reduce(
                        am_h[:], src[:], axis=X_AX, op=MAX, apply_absolute_value=True
                    )
                    mss.append(ms_h)
                    amaxs.append(am_h)
                ms = sp.tile([128, 1], f32, tag="sm", name="ms")
                nc.vector.tensor_add(ms[:], mss[0][:], mss[1][:])
                amax = sp.tile([128, 1], f32, tag="sm", name="amax")
                nc.vector.tensor_tensor(amax[:], amaxs[0][:], amaxs[1][:], op=MAX)
                # r = 1/sqrt(ms/D + eps)
                sdev = sp.tile([128, 1], f32, tag="sm", name="sdev")
                nc.scalar.activation(
                    sdev[:], ms[:], AF.Sqrt, bias=eps_rep[:], scale=1.0 / D
                )
                r = sp.tile([128, 1], f32, tag="sm", name="r")
                nc.vector.reciprocal(r[:], sdev[:])
                # amc = max(amax*r, eps) = max(max|xn|, eps);  c = r*127/amc
                nc.vector.tensor_scalar(
                    amc_all[:, t : t + 1], amax[:], r[:], Q_EPS, op0=MULT, op1=MAX
                )
                inv = sp.tile([128, 1], f32, tag="sm", name="inv")
                nc.vector.reciprocal(inv[:], amc_all[:, t : t + 1])
                nc.vector.tensor_scalar(
                    c_all[:, t : t + 1], r[:], inv[:], 127.0, op0=MULT, op1=MULT
                )
                x_state[t] = srcs

            def emit_x_round(t):
                srcs = x_state.pop(t)
                sqs = []
                for h in range(2):
                    src = srcs[h]
                    if with_g:
                        sq = src  # in-place round on the x*g scratch
                    else:
                        sq = scr_x.tile([128, HALF], f32, tag="scx", name="sq")
                    # v = src*c + C_ROUND  (fp32: rounds to int, ties-even)
                    nc.vector.tensor_scalar(
                        sq[:], src[:], c_all[:, t : t + 1], C_ROUND, op0=MULT, op1=ADD
                    )
                    sqs.append(sq)
                # transpose v (fp32) via PE; subtract C_ROUND during the
                # PSUM->SBUF copy (fp32->bf16): xqT gets exact int8. One of
                # four copies per half runs on DVE to balance the ACT load.
                for h in range(2):
                    sq = sqs[h]
                    for bk in range(4):
                        pst = ps_t.tile([128, 512], f32, tag="pst", name="pstx")
                        for j4 in range(4):
                            j = bk * 4 + j4
                            nc.tensor.transpose(
                                pst[:, ts(j4, 128)], sq[:, ts(j, 128)], ident[:]
                            )
                        dc0 = h * (HALF // 128) + bk * 4
                        dst = xqT[:, dc0 : dc0 + 4, ts(t, 128)]
                        psrc = pst[:].rearrange("p (j q) -> p j q", j=4)
                        if bk == 3:
                            nc.vector.tensor_scalar_add(dst, psrc, -C_ROUND)
                        else:
                            nc.scalar.activation(dst, psrc, AF.Copy, bias=-C_ROUND)

            # ---- Wq(c): stream chunk c of wt, quantize to ternary bf16
            def emit_wq(c):
                tiles = []
                for g in range(NST):
                    ws = wst.tile([128, SDC, 512], f32, tag="wst", name="ws_t")
                    src = wtp[ts(g, SDC * 128), ts(c, 512)].rearrange(
                        "(j p) q -> p j q", p=128
                    )
                    nc.sync.dma_start(ws[:], src)
                    flat = ws[:].rearrange("p j q -> p (j q)")
                    # v = w*wsc + C_ROUND (fp32 round); wq = Sign(v - C_ROUND)
                    nc.vector.tensor_scalar(
                        flat, flat, wsc_rep[:], C_ROUND, op0=MULT, op1=ADD
                    )
                    wq = wcb_pool.tile([128, SDC, 512], bf16, tag="wcb", name="wq")
                    nc.scalar.activation(
                        wq[:].rearrange("p j q -> p (j q)"),
                        flat,
                        AF.Sign,
                        bias=nround_rep[:],
                    )
                    tiles.append(wq)
                return tiles

            # ---- MM(c): 8 token tiles x 32 accumulating matmuls
            def emit_mm(c, wq_tiles):
                for t in range(NTT):
                    pmm = ps_mm.tile([128, 512], f32, tag="pmm", name="pmm")
                    for g in range(NST):
                        for j in range(SDC):
                            dc = g * SDC + j
                            nc.tensor.matmul(
                                pmm[:],
                                lhsT=xqT[:, dc, ts(t, 128)],
                                rhs=wq_tiles[g][:, j, :],
                                start=(dc == 0),
                                stop=(dc == NDC - 1),
                            )
                    y_sb = ypool.tile([128, 512], f32, tag="y", name="y_sb")
                    nc.vector.tensor_scalar_mul(
                        y_sb[:], pmm[:], c2_all[:, t : t + 1]
                    )
                    nc.sync.dma_start(yp[ts(t, 128), ts(c, 512)], y_sb[:])

            # ================= emission schedule =================
            # Software-pipelined x-prep: stats(t) is emitted xpre tiles
            # ahead of round(t) so no engine FIFO serializes on a
            # cross-engine dependency chain within one tile. W1 DMAs are
            # interleaved so the x0 DMA isn't queued behind all 8.4MB of
            # the wsl read.
            emit_x_stats(0)
            for ih in range(4):
                emit_w1(ih // 2, ih % 2)
            if xpre > 1:
                emit_x_stats(1)
            for ih in range(4, 8):
                emit_w1(ih // 2, ih % 2)

            # finish the abs-sum -> scalar AllReduce. Emitted two x-tiles
            # into the pipeline: the DVE tree-adds wait on the ACT abs-sums
            # (~40us) and would otherwise head-of-line-block the first x
            # round passes in the DVE FIFO.
            def emit_ar_finish():
                ps = parts
                while len(ps) > 1:
                    nxt = []
                    for a, b_ in zip(ps[::2], ps[1::2]):
                        s2 = sp.tile([128, 1], f32, tag="sm", name="s2")
                        nc.vector.tensor_add(s2[:], a[:], b_[:])
                        nxt.append(s2)
                    if len(ps) % 2:
                        nxt.append(ps[-1])
                    ps = nxt
                pst_s = ps_mm.tile([128, 512], f32, tag="pmm", name="pmm")
                nc.tensor.matmul(
                    pst_s[:1, :1], lhsT=ps[0][:], rhs=ones_col[:], start=True,
                    stop=True,
                )
                sb_tot = sp.tile([1, 1], f32, tag="one", name="sb_tot")
                nc.scalar.copy(sb_tot[:], pst_s[:1, :1])
                nc.sync.dma_start(ws_in[:], sb_tot[:])
                nc.gpsimd.collective_compute(
                    "AllReduce",
                    ADD,
                    replica_groups=GROUP,
                    ins=[ws_in[:]],
                    outs=[ws_out[:]],
                )
                nc.sync.dma_start(s_rep[:], ws_out[:].to_broadcast([128, 1]))

            for t in range(xpre, NTT):
                emit_x_round(t - xpre)
                emit_x_stats(t)
                if t == 3:
                    emit_ar_finish()
            for t in range(NTT - xpre, NTT):
                emit_x_round(t)

            # w_scale machinery (replicated per partition):
            #   m_rep  = max(mean|W|, Q_EPS)   (= 1/w_scale)
            #   wsc_rep= 1/m_rep               (= w_scale)
            #   dq_rep = m_rep/127             (= 1/(127*w_scale))
            # tile_wait_until: keep every AR-gated op AFTER all x-prep work
            # in the static per-engine order, so the AR sem-wait can't
            # head-of-line-block the x5-7 quantize on DVE/ACT.
            with tc.tile_wait_until(0.16):
                nc.vector.tensor_scalar(
                    m_rep[:], s_rep[:], 1.0 / (N * D), Q_EPS, op0=MULT, op1=MAX
                )
                nc.vector.reciprocal(wsc_rep[:], m_rep[:])
                nc.vector.tensor_scalar_mul(dq_rep[:], m_rep[:], 1.0 / 127.0)
                # c2 = 1/(w_scale*x_scale) per token tile
                for t in range(NTT):
                    nc.vector.tensor_mul(
                        c2_all[:, t : t + 1], amc_all[:, t : t + 1], dq_rep[:]
                    )

                wq_tiles = [emit_wq(0), emit_wq(1)]
            for c in range(NCH):
                emit_mm(c, wq_tiles[c])
                if c + 2 < NCH:
                    wq_tiles.append(emit_wq(c + 2))

    orig = nc.to_json_bytes

    def patched():
        return _legalize_waits(orig())

    nc.to_json_bytes = patched
    return nc


def _get_nc(with_g):
    key = ("nc", with_g)
    if key not in _CACHED:
        _CACHED[key] = _build(with_g)
    return _CACHED[key]


def make_in_maps(x, weight, norm_weight):
    x = np.ascontiguousarray(x, dtype=np.float32)
    weight = np.ascontiguousarray(weight, dtype=np.float32)
    norm_weight = np.ascontiguousarray(norm_weight, dtype=np.float32)
    xf = x.reshape(B * S, D)
    wt = np.ascontiguousarray(weight.T)
    in_maps = []
    for i in range(R):
        in_maps.append(
            {
                "x": xf[i * TOK : (i + 1) * TOK],
                "wt": wt,
                "wsl": wt[i * NS : (i + 1) * NS],
                "g": norm_weight.reshape(1, D),
            }
        )
    return in_maps


def kernel(x, weight, norm_weight):
    from concourse.bass_utils import run_bass_kernel_spmd

    in_maps = make_in_maps(x, weight, norm_weight)
    with_g = not bool(np.all(np.asarray(norm_weight) == 1.0))
    nc = _get_nc(with_g)
    res = run_bass_kernel_spmd(nc, in_maps, list(range(R)))
    y = np.concatenate([res.results[i]["y"] for i in range(R)], axis=0)
    return y.reshape(B, S, N)


if __name__ == "__main__":
    rng = np.random.default_rng(0)
    x = rng.standard_normal((B, S, D), dtype=np.float32)
    w = (rng.standard_normal((N, D), dtype=np.float32) * np.sqrt(2.0 / D)).astype(
        np.float32
    )
    g = np.ones(D, dtype=np.float32)
    y = kernel(x, w, g)
    print("ran", y.shape, y.dtype)


# revision 47
# speedup vs baseline: 1.1606x; 1.1606x over previous
"""BitLinear forward on 8 Trainium2 NeuronCores.

Reference computation (see harness reference.py):
    xn      = rmsnorm(x) * norm_weight                     # per token over D
    w_scale = 1 / max(mean(|W|), 1e-5)                     # global scalar
    w_q     = clip(round(W * w_scale), -1, 1)              # ternary
    x_scale = 127 / max(max|xn| per token, 1e-5)
    x_q     = clip(round(xn * x_scale), -128, 127)
    y       = (x_q @ w_q.T) / (w_scale * x_scale)

Distribution: data-parallel over tokens (1024/core), weight REPLICATED.
The host passes W pre-transposed (wt = W.T, [D, N] row-major) to every
core, so each core:
  - computes the global |W| abs-sum from its own disjoint 512-row slice
    of wt, AllReduces the scalar (the only collective on the critical
    path; a tiny warm-up AllReduce is issued at kernel start so the
    real one doesn't pay collective cold-start / launch skew),
  - rmsnorms + int8-quantizes + PE-transposes its 1024 tokens into a
    resident xqT [128, 32dc, 1024tok] bf16 SBUF tile, all before the
    matmul phase starts - no mid-matmul x stalls,
  - streams wt from its own DRAM in [128, 4dc, 512out] fp32 stages,
    quantizing to ternary bf16 on the fly (DVE round via the fp32
    +1.5*2^23 trick, then ACT Sign: clip(round(v),-1,1) == sign of the
    rounded integer; all ACT funcs used live in one table set),
  - runs 8 out-chunks x 8 token-tiles x 32 accumulating bf16 matmuls
    (exact integer arithmetic: x_q in [-127,127], w_q in {-1,0,1},
    fp32 PSUM partial sums < 2^24).
No AllGather, no quantized-W DRAM round-trip.
"""

import numpy as np

# ---------------------------------------------------------------- constants
R = 8  # cores
B, S, D = 4, 2048, 4096
N = 4096  # out features
TOK = (B * S) // R  # tokens per core (1024)
NS = N // R  # wslice rows per core (512)
HALF = D // 2  # x free-dim half tile (2048)
NTT = TOK // 128  # token tiles per core (8)
NDC = D // 128  # contraction chunks (32)
NCH = N // 512  # output chunks (8)
NST = 8  # W stages per chunk (4 dc each)
SDC = NDC // NST  # dc per stage (4)
C_ROUND = 12582912.0  # 1.5 * 2^23: fp32 add rounds to int, ties-to-even
EPS_NORM = 1e-5
Q_EPS = 1e-5

_CACHED = {}


def _legalize_waits(bir_bytes):
    """Split multi-wait BIR instructions into single-wait EventSemaphore
    chains: the walrus build here accepts at most one sync-wait command per
    instruction, while Tile's sem-assignment emits multi-wait joins."""
    import json

    bir = json.loads(bir_bytes)
    for fn in bir.get("functions", []):
        for bb in fn.get("blocks", []):
            new_insts = []
            for inst in bb.get("instructions", []):
                si = inst.get("sync_info")
                waits = (si or {}).get("on_wait") or []
                if len(waits) > 1:
                    movable = [w for w in waits if w.get("sync_type") == "semaphore"]
                    fixed = [w for w in waits if w.get("sync_type") != "semaphore"]
                    keep, hoist = (
                        (fixed, movable) if fixed else ([movable[-1]], movable[:-1])
                    )
                    if len(keep) > 1:
                        raise RuntimeError(
                            f"{inst.get('name')}: {len(keep)} non-hoistable waits"
                        )
                    for k, w in enumerate(hoist):
                        new_insts.append(
                            {
                                "debug": inst.get("debug", 0),
                                "engine": inst["engine"],
                                "ins": [],
                                "name": f"{inst['name']}_hw{k}",
                                "opcode": "EventSemaphore",
                                "outs": [],
                                "sync_info": {"on_update": [], "on_wait": [w]},
                            }
                        )
                    si["on_wait"] = keep
                new_insts.append(inst)
            bb["instructions"] = new_insts
    return json.dumps(bir).encode()


def _build(with_g):
    import concourse.bass as bass
    import concourse.mybir as mybir
    import concourse.tile as tile
    from concourse.bass import ts
    from concourse.masks import make_identity

    f32 = mybir.dt.float32
    bf16 = mybir.dt.bfloat16
    MULT = mybir.AluOpType.mult
    ADD = mybir.AluOpType.add
    MAX = mybir.AluOpType.max
    X_AX = mybir.AxisListType.X
    AF = mybir.ActivationFunctionType
    GROUP = [list(range(R))]

    nc = bass.Bass()
    xp = nc.declare_dram_parameter("x", [TOK, D], f32, isOutput=False)
    wtp = nc.declare_dram_parameter("wt", [D, N], f32, isOutput=False)
    wsl = nc.declare_dram_parameter("wsl", [NS, D], f32, isOutput=False)
    gp = nc.declare_dram_parameter("g", [1, D], f32, isOutput=False)
    yp = nc.declare_dram_parameter("y", [TOK, N], f32, isOutput=True)

    wcb_bufs = 12 if with_g else 16
    iox_bufs = 2 if with_g else 4
    scr_bufs = 4 if with_g else 2
    # software-pipeline depth of the x-prep: how many stats blocks are
    # emitted ahead of the trailing round/transpose blocks
    xpre = 1 if with_g else 2

    with tile.TileContext(nc) as tc:
        with (
            tc.tile_pool(name="persist", bufs=1) as pp,
            tc.tile_pool(name="io_x", bufs=iox_bufs) as io_x,
            tc.tile_pool(name="scr_x", bufs=scr_bufs) as scr_x,
            tc.tile_pool(name="wst", bufs=2) as wst,
            tc.tile_pool(name="wcb", bufs=wcb_bufs) as wcb_pool,
            tc.tile_pool(name="small", bufs=24) as sp,
            tc.tile_pool(name="yout", bufs=2) as ypool,
            tc.tile_pool(name="ps_t", bufs=2, space="PSUM") as ps_t,
            tc.tile_pool(name="ps_mm", bufs=6, space="PSUM") as ps_mm,
            tc.tile_pool(name="dram", bufs=1, space="DRAM") as dram,
        ):
            # ---- persistent tiles
            xqT = pp.tile([128, NDC, TOK], bf16, name="xqT")
            ident = pp.tile([128, 128], f32, name="ident")
            make_identity(nc, ident[:])
            ones_col = pp.tile([128, 1], f32, name="ones_col")
            nc.vector.memset(ones_col[:], 1.0)
            amc_all = pp.tile([128, NTT], f32, name="amc_all")
            c2_all = pp.tile([128, NTT], f32, name="c2_all")
            c_all = pp.tile([128, NTT], f32, name="c_all")
            s_rep = pp.tile([128, 1], f32, name="s_rep")
            m_rep = pp.tile([128, 1], f32, name="m_rep")
            wsc_rep = pp.tile([128, 1], f32, name="wsc_rep")
            dq_rep = pp.tile([128, 1], f32, name="dq_rep")
            eps_rep = pp.tile([128, 1], f32, name="eps_rep")
            nc.vector.memset(eps_rep[:], EPS_NORM)
            nround_rep = pp.tile([128, 1], f32, name="nround_rep")
            nc.vector.memset(nround_rep[:], -C_ROUND)
            if with_g:
                g_rep = pp.tile([128, D], f32, name="g_rep")
                nc.sync.dma_start(g_rep[:], gp[:].to_broadcast([128, D]))

            # ---- DRAM scratch
            ws_in = dram.tile([1, 1], f32, name="ws_in")
            ws_out = dram.tile([1, 1], f32, addr_space="Shared", name="ws_out")

            # ---- W1: partial |W| abs-sum over this core's disjoint slice
            parts = []

            def emit_w1(i, h):
                w_t = wst.tile([128, SDC, 512], f32, tag="wst", name="ws_t")
                fl = w_t[:].rearrange("p j q -> p (j q)")
                nc.sync.dma_start(fl, wsl[ts(i, 128), ts(h, HALF)])
                part = sp.tile([128, 1], f32, tag="sm", name="part")
                nc.scalar.activation(fl, fl, AF.Abs, accum_out=part[:])
                parts.append(part)

            # ---- X(t): rmsnorm + int8 quantize + transpose into xqT.
            # Split into a stats part and a round/transpose part so the
            # emission loop can software-pipeline them (stats of tile t+1
            # ahead of round of tile t in every engine FIFO).
            x_state = {}

            def emit_x_stats(t):
                srcs = []
                mss, amaxs = [], []
                for h in range(2):
                    x_t = io_x.tile([128, HALF], f32, tag="iox", name="x_t")
                    nc.sync.dma_start(x_t[:], xp[ts(t, 128), ts(h, HALF)])
                    ms_h = sp.tile([128, 1], f32, tag="sm", name="ms_h")
                    sq = scr_x.tile([128, HALF], f32, tag="scx", name="sq")
                    # sq <- x*x (junk scratch), ms_h <- sum(x^2)
                    nc.scalar.activation(sq[:], x_t[:], AF.Square, accum_out=ms_h[:])
                    if with_g:
                        nc.vector.tensor_mul(sq[:], x_t[:], g_rep[:, ts(h, HALF)])
                        src = sq
                    else:
                        src = x_t
                    srcs.append(src)
                    am_h = sp.tile([128, 1], f32, tag="sm", name="am_h")
                    nc.vector.tensor_reduce(
                        am_h[:], src[:], axis=X_AX, op=MAX, apply_absolute_value=True
                    )
                    mss.append(ms_h)
                    amaxs.append(am_h)
                ms = sp.tile([128, 1], f32, tag="sm", name="ms")
                nc.vector.tensor_add(ms[:], mss[0][:], mss[1][:])
                amax = sp.tile([128, 1], f32, tag="sm", name="amax")
                nc.vector.tensor_tensor(amax[:], amaxs[0][:], amaxs[1][:], op=MAX)
                # r = 1/sqrt(ms/D + eps)
                sdev = sp.tile([128, 1], f32, tag="sm", name="sdev")
                nc.scalar.activation(
                    sdev[:], ms[:], AF.Sqrt, bias=eps_rep[:], scale=1.0 / D
                )
                r = sp.tile([128, 1], f32, tag="sm", name="r")
                nc.vector.reciprocal(r[:], sdev[:])
                # amc = max(amax*r, eps) = max(max|xn|, eps);  c = r*127/amc
                nc.vector.tensor_scalar(
                    amc_all[:, t : t + 1], amax[:], r[:], Q_EPS, op0=MULT, op1=MAX
                )
                inv = sp.tile([128, 1], f32, tag="sm", name="inv")
                nc.vector.reciprocal(inv[:], amc_all[:, t : t + 1])
                nc.vector.tensor_scalar(
                    c_all[:, t : t + 1], r[:], inv[:], 127.0, op0=MULT, op1=MULT
                )
                x_state[t] = srcs

            def emit_x_round(t):
                srcs = x_state.pop(t)
                sqs = []
                for h in range(2):
                    src = srcs[h]
                    if with_g:
                        sq = src  # in-place round on the x*g scratch
                    else:
                        sq = scr_x.tile([128, HALF], f32, tag="scx", name="sq")
                    # v = src*c + C_ROUND  (fp32: rounds to int, ties-even)
                    nc.vector.tensor_scalar(
                        sq[:], src[:], c_all[:, t : t + 1], C_ROUND, op0=MULT, op1=ADD
                    )
                    sqs.append(sq)
                # transpose v (fp32) via PE; subtract C_ROUND during the
                # PSUM->SBUF copy (fp32->bf16): xqT gets exact int8. One of
                # four copies per half runs on DVE to balance the ACT load.
                for h in range(2):
                    sq = sqs[h]
                    for bk in range(4):
                        pst = ps_t.tile([128, 512], f32, tag="pst", name="pstx")
                        for j4 in range(4):
                            j = bk * 4 + j4
                            nc.tensor.transpose(
                                pst[:, ts(j4, 128)], sq[:, ts(j, 128)], ident[:]
                            )
                        dc0 = h * (HALF // 128) + bk * 4
                        dst = xqT[:, dc0 : dc0 + 4, ts(t, 128)]
                        psrc = pst[:].rearrange("p (j q) -> p j q", j=4)
                        if bk == 3:
                            nc.vector.tensor_scalar_add(dst, psrc, -C_ROUND)
                        else:
                            nc.scalar.activation(dst, psrc, AF.Copy, bias=-C_ROUND)

            # ---- Wq(c): stream chunk c of wt, quantize to ternary bf16
            def emit_wq(c):
                tiles = []
                for g in range(NST):
                    ws = wst.tile([128, SDC, 512], f32, tag="wst", name="ws_t")
                    src = wtp[ts(g, SDC * 128), ts(c, 512)].rearrange(
                        "(j p) q -> p j q", p=128
                    )
                    nc.sync.dma_start(ws[:], src)
                    flat = ws[:].rearrange("p j q -> p (j q)")
                    # v = w*wsc + C_ROUND (fp32 round); wq = Sign(v - C_ROUND)
                    nc.vector.tensor_scalar(
                        flat, flat, wsc_rep[:], C_ROUND, op0=MULT, op1=ADD
                    )
                    wq = wcb_pool.tile([128, SDC, 512], bf16, tag="wcb", name="wq")
                    nc.scalar.activation(
                        wq[:].rearrange("p j q -> p (j q)"),
                        flat,
                        AF.Sign,
                        bias=nround_rep[:],
                    )
                    tiles.append(wq)
                return tiles

            # ---- MM(c): 8 token tiles x 32 accumulating matmuls
            def emit_mm(c, wq_tiles):
                for t in range(NTT):
                    pmm = ps_mm.tile([128, 512], f32, tag="pmm", name="pmm")
                    for g in range(NST):
                        for j in range(SDC):
                            dc = g * SDC + j
                            nc.tensor.matmul(
                                pmm[:],
                                lhsT=xqT[:, dc, ts(t, 128)],
                                rhs=wq_tiles[g][:, j, :],
                                start=(dc == 0),
                                stop=(dc == NDC - 1),
                            )
                    y_sb = ypool.tile([128, 512], f32, tag="y", name="y_sb")
                    nc.vector.tensor_scalar_mul(
                        y_sb[:], pmm[:], c2_all[:, t : t + 1]
                    )
                    nc.sync.dma_start(yp[ts(t, 128), ts(c, 512)], y_sb[:])

            # ================= emission schedule =================
            # Software-pipelined x-prep: stats(t) is emitted xpre tiles
            # ahead of round(t) so no engine FIFO serializes on a
            # cross-engine dependency chain within one tile. W1 DMAs are
            # interleaved so the x0 DMA isn't queued behind all 8.4MB of
            # the wsl read.
            emit_x_stats(0)
            for ih in range(4):
                emit_w1(ih // 2, ih % 2)
            if xpre > 1:
                emit_x_stats(1)
            for ih in range(4, 8):
                emit_w1(ih // 2, ih % 2)

            # finish the abs-sum -> scalar AllReduce. Emitted two x-tiles
            # into the pipeline: the DVE tree-adds wait on the ACT abs-sums
            # (~40us) and would otherwise head-of-line-block the first x
            # round passes in the DVE FIFO.
            def emit_ar_finish():
                ps = parts
                while len(ps) > 1:
                    nxt = []
                    for a, b_ in zip(ps[::2], ps[1::2]):
                        s2 = sp.tile([128, 1], f32, tag="sm", name="s2")
                        # a+b on ACT (Identity with AP bias): keeps the
                        # abs-sum tree off the DVE FIFO, where it would
                        # head-of-line-block x round passes behind the
                        # ACT abs-sum completion
                        nc.scalar.activation(
                            s2[:], a[:], AF.Identity, bias=b_[:]
                        )
                        nxt.append(s2)
                    if len(ps) % 2:
                        nxt.append(ps[-1])
                    ps = nxt
                pst_s = ps_mm.tile([128, 512], f32, tag="pmm", name="pmm")
                nc.tensor.matmul(
                    pst_s[:1, :1], lhsT=ps[0][:], rhs=ones_col[:], start=True,
                    stop=True,
                )
                sb_tot = sp.tile([1, 1], f32, tag="one", name="sb_tot")
                nc.scalar.copy(sb_tot[:], pst_s[:1, :1])
                nc.sync.dma_start(ws_in[:], sb_tot[:])
                nc.gpsimd.collective_compute(
                    "AllReduce",
                    ADD,
                    replica_groups=GROUP,
                    ins=[ws_in[:]],
                    outs=[ws_out[:]],
                )
                nc.sync.dma_start(s_rep[:], ws_out[:].to_broadcast([128, 1]))

            emit_ar_finish()
            for t in range(xpre, NTT):
                emit_x_round(t - xpre)
                emit_x_stats(t)
            for t in range(NTT - xpre, NTT):
                emit_x_round(t)

            # w_scale machinery (replicated per partition):
            #   m_rep  = max(mean|W|, Q_EPS)   (= 1/w_scale)
            #   wsc_rep= 1/m_rep               (= w_scale)
            #   dq_rep = m_rep/127             (= 1/(127*w_scale))
            # tile_wait_until: keep every AR-gated op AFTER all x-prep work
            # in the static per-engine order, so the AR sem-wait can't
            # head-of-line-block the x5-7 quantize on DVE/ACT.
            with tc.tile_wait_until(0.16):
                nc.vector.tensor_scalar(
                    m_rep[:], s_rep[:], 1.0 / (N * D), Q_EPS, op0=MULT, op1=MAX
                )
                nc.vector.reciprocal(wsc_rep[:], m_rep[:])
                nc.vector.tensor_scalar_mul(dq_rep[:], m_rep[:], 1.0 / 127.0)
                # c2 = 1/(w_scale*x_scale) per token tile
                for t in range(NTT):
                    nc.vector.tensor_mul(
                        c2_all[:, t : t + 1], amc_all[:, t : t + 1], dq_rep[:]
                    )

                wq_tiles = [emit_wq(0), emit_wq(1)]
            for c in range(NCH):
                emit_mm(c, wq_tiles[c])
                if c + 2 < NCH:
                    wq_tiles.append(emit_wq(c + 2))

    orig = nc.to_json_bytes

    def patched():
        return _legalize_waits(orig())

    nc.to_json_bytes = patched
    return nc


def _get_nc(with_g):
    key = ("nc", with_g)
    if key not in _CACHED:
        _CACHED[key] = _build(with_g)
    return _CACHED[key]


def make_in_maps(x, weight, norm_weight):
    x = np.ascontiguousarray(x, dtype=np.float32)
    weight = np.ascontiguousarray(weight, dtype=np.float32)
    norm_weight = np.ascontiguousarray(norm_weight, dtype=np.float32)
    xf = x.reshape(B * S, D)
    wt = np.ascontiguousarray(weight.T)
    in_maps = []
    for i in range(R):
        in_maps.append(
            {
                "x": xf[i * TOK : (i + 1) * TOK],
                "wt": wt,
                "wsl": wt[i * NS : (i + 1) * NS],
                "g": norm_weight.reshape(1, D),
            }
        )
    return in_maps


def kernel(x, weight, norm_weight):
    from concourse.bass_utils import run_bass_kernel_spmd

    in_maps = make_in_maps(x, weight, norm_weight)
    with_g = not bool(np.all(np.asarray(norm_weight) == 1.0))
    nc = _get_nc(with_g)
    res = run_bass_kernel_spmd(nc, in_maps, list(range(R)))
    y = np.concatenate([res.results[i]["y"] for i in range(R)], axis=0)
    return y.reshape(B, S, N)


if __name__ == "__main__":
    rng = np.random.default_rng(0)
    x = rng.standard_normal((B, S, D), dtype=np.float32)
    w = (rng.standard_normal((N, D), dtype=np.float32) * np.sqrt(2.0 / D)).astype(
        np.float32
    )
    g = np.ones(D, dtype=np.float32)
    y = kernel(x, w, g)
    print("ran", y.shape, y.dtype)
